# revision 39
# baseline (speedup 1.0000x reference)
"""Causal self-attention with RoPE on 8 Trainium2 NeuronCores.

Strategy (tensor-parallel over heads, SPMD-uniform, collective-free):
  - 12 heads -> 8 cores x 2 head slots (4 slots get zero weights).
  - Per core: QKV projection for its 2 heads in [channel, seq] layout;
    RoPE via 3 DVE tensor-tensor ops with sign-folded cos/sin tables;
    causal flash-style attention with scores kept transposed
    (S^T[keys, queries]) so P tiles feed the AV matmul directly; softmax
    denominators via a ones-column in V; per-head normalization; partial
    output projection through the core's slice of w_proj columns.
  - QKV chunk J is interleaved with attention chunk J (causality means
    chunk J only attends keys < 512(J+1)), keeping ScalarE (exp) busy
    from the start.
  - The 8-way sum of partial projections runs on-device: each chunk's
    [C, 512] fp16 partial is ReduceScattered across the cores (overlapped
    with the next chunk's compute), so each core ships back only its
    disjoint [C/8, T] slice.  Host concatenates + transposes.  All
    matmuls run in float32r (TF32-like).
  - Dispatch uses fast_dispatch_compile (no-effect C++ path) on a
    jitted shard_map over the 8 axon-tunneled cores.
"""

import sys

sys.path.insert(0, "/opt/trn_rl_repo")

import numpy as np

import concourse.bass as bass
import concourse.mybir as mybir
import concourse.tile as tile
from concourse import bacc, bass2jax
from concourse.masks import make_identity

FP32 = mybir.dt.float32
FP32R = mybir.dt.float32r
FP16 = mybir.dt.float16
AF = mybir.ActivationFunctionType
ALU = mybir.AluOpType

T = 4096
C = 768
D = 64
N_HEAD = 12
N_CORES = 8
CHUNK = 512          # query chunk (matmul free dim)
NCHUNK = T // CHUNK  # 8
KT = 128             # key tile
ROPE_BASE = 10000.0

# core -> (head_slot_a, head_slot_b); None = zero slot
HEAD_MAP = [(0, 8), (1, 9), (2, 10), (3, 11),
            (4, None), (5, None), (6, None), (7, None)]

_PROG = None  # cached compiled program


def build_program():
    """Build + compile the per-core Bass program (identical on all cores)."""
    nc = bacc.Bacc("TRN2", target_bir_lowering=False, debug=False,
                   num_devices=N_CORES)

    xT_d = nc.dram_tensor("xT", [C, T], FP32R, kind="ExternalInput").ap()
    wqk_u_d = nc.dram_tensor("wqk_u", [C, 256], FP32R, kind="ExternalInput").ap()
    wqk_w_d = nc.dram_tensor("wqk_w", [C, 256], FP32R, kind="ExternalInput").ap()
    w_v_d = nc.dram_tensor("w_v", [C, 128], FP32R, kind="ExternalInput").ap()
    w_pT_d = nc.dram_tensor("w_projT", [128, C], FP32R, kind="ExternalInput").ap()
    cos_d = nc.dram_tensor("rope_cos", [128, T], FP32, kind="ExternalInput").ap()
    sin_d = nc.dram_tensor("rope_sin", [128, T], FP32, kind="ExternalInput").ap()
    mask_d = nc.dram_tensor("masks", [128, 4 * CHUNK], FP32R, kind="ExternalInput").ap()
    CS = C // N_CORES  # 96 rows of the reduced output owned by this core
    out_d = nc.dram_tensor("outT", [CS, T], FP16, kind="ExternalOutput").ap()

    with tile.TileContext(nc) as tc:
        with (
            tc.tile_pool(name="persist", bufs=1) as pers,
            tc.tile_pool(name="xin", bufs=3) as xin,
            tc.tile_pool(name="tmp", bufs=4) as tmps,
            tc.tile_pool(name="ptile", bufs=8) as ptile,
            tc.tile_pool(name="ostage", bufs=4) as ostage,
            tc.tile_pool(name="small", bufs=6) as small,
            tc.tile_pool(name="dram", bufs=1, space="DRAM") as dram,
            tc.tile_pool(name="psUW", bufs=2, space="PSUM") as psUW,  # qkv accumulators
            tc.tile_pool(name="psS", bufs=3, space="PSUM") as psS,    # score tiles
            tc.tile_pool(name="psY", bufs=1, space="PSUM") as psY,    # y accum
            tc.tile_pool(name="psX", bufs=1, space="PSUM") as psX,    # aux (pb/tr) + o
        ):
            # per-chunk partial projections + reduce-scatter bounce buffers
            part = dram.tile([NCHUNK, C, CHUNK], FP16)
            rs_out = dram.tile([NCHUNK, CS, CHUNK], FP16)
            # ---- persistent SBUF ----
            wqk_u = pers.tile([128, 6, 256], FP32R)
            wqk_w = pers.tile([128, 6, 256], FP32R)
            w_v = pers.tile([128, 6, 128], FP32R)
            w_pT = pers.tile([128, C], FP32R)

            mask_sb = pers.tile([128, 4 * CHUNK], FP32R)
            QT = pers.tile([128, T], FP32R)   # rows 0-63 head A, 64-127 head B
            KTt = pers.tile([128, T], FP32R)
            V = pers.tile([128, 32, 130], FP32R)  # [key%128, keytile, vA|1|vB|1]
            Y = pers.tile([128, T], FP32R)    # normalized attention out [ych, q]
            ones_sb = pers.tile([128, D], FP32R)
            ident = pers.tile([128, 128], FP32)
            make_identity(nc, ident[:])

            nc.sync.dma_start(wqk_u[:], wqk_u_d.rearrange("(o p) m -> p o m", p=128))
            nc.gpsimd.dma_start(w_v[:], w_v_d.rearrange("(o p) m -> p o m", p=128))
            nc.gpsimd.dma_start(w_pT[:], w_pT_d[:])
            nc.gpsimd.dma_start(mask_sb[:], mask_d[:])
            ones_f32 = pers.tile([128, D], FP32)
            nc.any.memset(ones_f32[:], 1.0)
            # HAM warm-up: keep PE busy during the initial input DMAs so the
            # clock gate reaches 8/8 before the first real matmuls (results
            # discarded; the tiny copy keeps DCE from dropping the chain)
            warm_ps = psS.tile([128, 128], FP32, tag="s")
            for i in range(16):
                nc.tensor.matmul(warm_ps[:], ident[:, 0:128], ident[:, 0:128],
                                 start=True, stop=True)
            warm_sb = small.tile([1, 8], FP32, tag="warm")
            nc.vector.tensor_copy(warm_sb[:], warm_ps[0:1, 0:8])
            nc.vector.tensor_copy(ones_sb[:], ones_f32[:])
            nc.vector.tensor_copy(V[:, :, 64], ones_sb[:, 0:32])
            nc.vector.tensor_copy(V[:, :, 129], ones_sb[:, 0:32])

            def emit_qkv(J):
                cols = slice(J * CHUNK, (J + 1) * CHUNK)
                xt = xin.tile([128, 6, CHUNK], FP32R)
                xT_r = xT_d.rearrange("(o p) n -> p o n", p=128)
                nc.sync.dma_start(xt[:, 0:3, :], xT_r[:, 0:3, cols])
                nc.sync.dma_start(xt[:, 3:6, :], xT_r[:, 3:6, cols])
                if J == 0:
                    nc.sync.dma_start(
                        wqk_w[:], wqk_w_d.rearrange("(o p) m -> p o m", p=128))
                cs_sb = tmps.tile([128, CHUNK], FP32, tag="cs")
                sn_sb = tmps.tile([128, CHUNK], FP32, tag="sn")
                nc.sync.dma_start(cs_sb[:], cos_d[:, cols])
                nc.sync.dma_start(sn_sb[:], sin_d[:, cols])
                for qk, tgt in ((0, QT), (1, KTt)):
                    u_ps = psUW.tile([128, CHUNK], FP32, tag="uw")
                    w_ps = psUW.tile([128, CHUNK], FP32, tag="uw")
                    wcol = slice(qk * 128, qk * 128 + 128)
                    for k in range(6):
                        nc.tensor.matmul(u_ps[:], wqk_u[:, k, wcol], xt[:, k, :],
                                         start=(k == 0), stop=(k == 5))
                    for k in range(6):
                        nc.tensor.matmul(w_ps[:], wqk_w[:, k, wcol], xt[:, k, :],
                                         start=(k == 0), stop=(k == 5))
                    tm = tmps.tile([128, CHUNK], FP32R, tag="ropetmp")
                    nc.vector.tensor_tensor(tgt[:, cols], u_ps[:], cs_sb[:], ALU.mult)
                    nc.vector.tensor_tensor(tm[:], w_ps[:], sn_sb[:], ALU.mult)
                    nc.vector.tensor_tensor(tgt[:, cols], tgt[:, cols], tm[:], ALU.add)
                # v in [ch, seq] (N=512 full-rate), then PE-transpose per key tile
                v_ps = psUW.tile([128, CHUNK], FP32, tag="uw")
                for k in range(6):
                    nc.tensor.matmul(v_ps[:], w_v[:, k, :], xt[:, k, :],
                                     start=(k == 0), stop=(k == 5))
                vT_sb = tmps.tile([128, CHUNK], FP32, tag="vt")
                nc.vector.tensor_copy(vT_sb[:], v_ps[:])
                for s in range(4):
                    kt_idx = 4 * J + s
                    tr_ps = psX.tile([128, 128], FP32, tag="aux")
                    nc.tensor.transpose(tr_ps[:], vT_sb[:, s * 128:(s + 1) * 128], ident)
                    nc.vector.tensor_copy(V[:, kt_idx, 0:64], tr_ps[:, 0:64])
                    nc.vector.tensor_copy(V[:, kt_idx, 65:129], tr_ps[:, 64:128])

            def emit_att(J):
                cols = slice(J * CHUNK, (J + 1) * CHUNK)
                nkt = 4 * J + 4
                for h in range(2):
                    hsl = slice(64 * h, 64 * h + 64)
                    vsl = slice(65 * h, 65 * h + 65)
                    y_ps = psY.tile([65, CHUNK], FP32, tag="y")
                    for t in range(nkt):
                        d = t - 4 * J
                        qlo = max(0, 128 * d)   # cols < qlo have no valid keys in tile t
                        sub = slice(qlo, CHUNK)
                        qsub = slice(J * CHUNK + qlo, (J + 1) * CHUNK)
                        s_ps = psS.tile([128, CHUNK], FP32, tag="s")
                        nc.tensor.matmul(
                            s_ps[:, sub], KTt[hsl, t * KT:(t + 1) * KT], QT[hsl, qsub],
                            start=True, stop=True)
                        p_sb = ptile.tile([128, CHUNK], FP32R, tag="p")
                        nc.scalar.activation(p_sb[:, sub], s_ps[:, sub], AF.Exp, scale=0.125)
                        if d >= 0:
                            # only qq in [qlo, qlo+128) straddles the causal
                            # boundary; columns beyond are fully valid
                            msub = slice(qlo, qlo + KT)
                            nc.vector.tensor_tensor(
                                p_sb[:, msub], p_sb[:, msub],
                                mask_sb[:, d * CHUNK + qlo:d * CHUNK + qlo + KT],
                                ALU.mult)
                        nc.tensor.matmul(
                            y_ps[:, sub], V[:, t, vsl], p_sb[:, sub],
                            start=(t == 0), stop=(t == nkt - 1))
                    rc = small.tile([1, CHUNK], FP32R, tag="rc")
                    with nc.allow_low_precision(reason="f32r recip for softmax denom"):
                        nc.vector.reciprocal(rc[0:1, :], y_ps[64:65, :])
                    rb = small.tile([64, CHUNK], FP32R, tag="rb")
                    nc.gpsimd.partition_broadcast(rb[:], rc[0:1, :])
                    nc.vector.tensor_tensor(Y[hsl, cols], y_ps[0:64, :], rb[:], ALU.mult)
                for m in range(6):
                    # last chunk: transposes are done, so borrow the aux slot
                    # to double-buffer the projection psum
                    otag = "aux" if (J == NCHUNK - 1 and m % 2) else "o"
                    o_ps = psX.tile([128, CHUNK], FP32, tag=otag)
                    nc.tensor.matmul(o_ps[:], w_pT[:, m * 128:(m + 1) * 128],
                                     Y[:, cols], start=True, stop=True)
                    o_sb = ostage.tile([128, CHUNK], FP16, tag="osb")
                    nc.vector.tensor_copy(o_sb[:], o_ps[:])
                    nc.sync.dma_start(part[J, m * 128:(m + 1) * 128, :], o_sb[:])
                # 8-way reduce-scatter of this chunk's [C, CHUNK] partial:
                # rank r keeps rows [96r, 96r+96).  Runs on the collective
                # engine, overlapped with the next chunk's compute.
                nc.gpsimd.collective_compute(
                    "ReduceScatter",
                    mybir.AluOpType.add,
                    replica_groups=[list(range(N_CORES))],
                    ins=[part[J].opt()],
                    outs=[rs_out[J].opt()],
                )
                nc.sync.dma_start(out_d[:, cols], rs_out[J])

            # attention J emitted right after its QKV; later QKV fills PE idle
            for J in range(NCHUNK):
                emit_qkv(J)
                emit_att(J)

    nc.compile()
    return nc


def _rope_tables():
    theta = 1.0 / (ROPE_BASE ** (np.arange(0, D, 2, dtype=np.float32) / D))  # [32]
    freqs = np.arange(T, dtype=np.float32)[None, :] * theta[:, None]  # [32, T]
    cos32 = np.cos(freqs).astype(np.float32)
    sin32 = np.sin(freqs).astype(np.float32)
    cos128 = np.tile(cos32, (4, 1))
    sin128 = np.concatenate([-sin32, sin32, -sin32, sin32], axis=0)
    return cos128, sin128


def _masks():
    m = np.zeros((128, 4 * CHUNK), dtype=np.float32)
    kk = np.arange(128)[:, None]
    qq = np.arange(CHUNK)[None, :]
    for d in range(4):
        m[:, d * CHUNK:(d + 1) * CHUNK] = (128 * d + kk <= qq).astype(np.float32)
    return m


def _swap_halves(w):
    # w: [rows multiple of 64, C]; swap 32-row halves within each 64 block
    r = w.reshape(-1, 2, 32, w.shape[-1])
    return r[:, ::-1].reshape(w.shape)


def make_in_maps(x, w_attn, w_proj):
    xT = np.ascontiguousarray(x.reshape(T, C).T)  # [C, T]
    cos128, sin128 = _rope_tables()
    masks = _masks()
    in_maps = []
    for c in range(N_CORES):
        qk_rows = []   # rows of w_attn for [qA, qB, kA, kB]
        v_rows = []    # [vA, vB]
        p_cols = []    # w_proj columns for [A(64), B(64)]
        sel = HEAD_MAP[c]
        for part_base in (0, C):  # q rows then k rows
            for h in sel:
                if h is None:
                    qk_rows.append(np.zeros((64, C), np.float32))
                else:
                    qk_rows.append(w_attn[part_base + 64 * h: part_base + 64 * h + 64])
        for h in sel:
            if h is None:
                v_rows.append(np.zeros((64, C), np.float32))
                p_cols.append(np.zeros((C, 64), np.float32))
            else:
                v_rows.append(w_attn[2 * C + 64 * h: 2 * C + 64 * h + 64])
                p_cols.append(w_proj[:, 64 * h: 64 * h + 64])
        qk = np.concatenate(qk_rows, axis=0)          # [256, C]
        wqk_u = np.ascontiguousarray(qk.T)            # [C, 256]
        wqk_w = np.ascontiguousarray(_swap_halves(qk).T)
        w_v = np.ascontiguousarray(np.concatenate(v_rows, axis=0).T)  # [C, 128]
        w_pT = np.ascontiguousarray(np.concatenate(p_cols, axis=1).T)  # [128, C]
        in_maps.append({
            "xT": xT, "wqk_u": wqk_u, "wqk_w": wqk_w, "w_v": w_v,
            "w_projT": w_pT, "rope_cos": cos128, "rope_sin": sin128,
            "masks": masks,
        })
    return in_maps


_RUNNER = None  # cached (jitted fn, metadata)


def build_runner(nc):
    """Jitted SPMD dispatch of the bass NEFF on 8 cores.

    The NEFF itself ends with an 8-way ReduceScatter, so each core returns
    only its disjoint [C/8, T] slice of the summed projection (12.6 MB total
    shipped back instead of 100 MB of per-core partials).
    """
    import jax
    from jax.sharding import Mesh, PartitionSpec
    from jax.experimental.shard_map import shard_map
    from concourse.bass2jax import _bass_exec_p, install_neuronx_cc_hook

    install_neuronx_cc_hook()
    partition_name = nc.partition_id_tensor.name if nc.partition_id_tensor else None
    in_names, in_shapes, out_names, out_avals, zero_outs = [], [], [], [], []
    for alloc in nc.m.functions[0].allocations:
        if not isinstance(alloc, mybir.MemoryLocationSet):
            continue
        name = alloc.memorylocations[0].name
        if alloc.kind == "ExternalInput":
            if name != partition_name:
                in_names.append(name)
                in_shapes.append((tuple(alloc.tensor_shape),
                                  mybir.dt.np(alloc.dtype)))
        elif alloc.kind == "ExternalOutput":
            np_dtype = mybir.dt.np(alloc.dtype)
            out_names.append(name)
            out_avals.append(jax.core.ShapedArray(tuple(alloc.tensor_shape),
                                                  np_dtype))
            zero_outs.append(np.zeros(tuple(alloc.tensor_shape), np_dtype))
    n_params = len(in_names)
    all_in_names = list(in_names) + list(out_names)
    if partition_name is not None:
        all_in_names.append(partition_name)

    def _body(*args):
        operands = list(args)
        if partition_name is not None:
            operands.append(bass2jax.partition_id_tensor())
        outs = _bass_exec_p.bind(
            *operands,
            out_avals=tuple(out_avals),
            in_names=tuple(all_in_names),
            out_names=tuple(out_names),
            lowering_input_output_aliases=(),
            sim_require_finite=True,
            sim_require_nnan=True,
            nc=nc,
        )
        return tuple(outs)

    devices = jax.devices()[:N_CORES]
    mesh = Mesh(np.asarray(devices), ("core",))
    in_specs = (PartitionSpec("core"),) * (n_params + len(out_names))
    sharding = jax.sharding.NamedSharding(mesh, PartitionSpec("core"))

    # Compile with the bass effect suppressed so dispatch takes JAX's C++
    # fast path (fast_dispatch_compile re-registers the error safety net).
    # Argument order matches put_args: inputs (in_names order), then the
    # zero-initialized output buffers.
    arg_structs = [
        jax.ShapeDtypeStruct((N_CORES * shape[0], *shape[1:]), dt,
                             sharding=sharding)
        for shape, dt in in_shapes
    ] + [
        jax.ShapeDtypeStruct((N_CORES * z.shape[0], *z.shape[1:]), z.dtype,
                             sharding=sharding)
        for z in zero_outs
    ]

    def compile_fn():
        jitted = jax.jit(
            shard_map(_body, mesh=mesh, in_specs=in_specs,
                      out_specs=PartitionSpec("core"), check_rep=False),
            keep_unused=True,
        )
        return jitted.lower(*arg_structs).compile()

    sharded = bass2jax.fast_dispatch_compile(compile_fn)

    def put_args(in_maps):
        import jax as _jax
        concat_in = [
            np.concatenate([np.asarray(in_maps[c][nm]) for c in range(N_CORES)],
                           axis=0)
            for nm in in_names
        ]
        concat_zero = [np.zeros((N_CORES * z.shape[0], *z.shape[1:]), z.dtype)
                       for z in zero_outs]
        return [_jax.device_put(a, sharding) for a in concat_in + concat_zero]

    return sharded, put_args


def get_runner():
    global _PROG, _RUNNER
    if _PROG is None:
        _PROG = build_program()
    if _RUNNER is None:
        _RUNNER = build_runner(_PROG)
    return _PROG, _RUNNER


def kernel(x, w_attn, w_proj):
    x = np.asarray(x, dtype=np.float32)
    w_attn = np.asarray(w_attn, dtype=np.float32)
    w_proj = np.asarray(w_proj, dtype=np.float32)
    _, (sharded, put_args) = get_runner()
    in_maps = make_in_maps(x, w_attn, w_proj)
    dev_args = put_args(in_maps)
    (outT,) = sharded(*dev_args)  # [C, T] fp16, already summed across cores
    outT = np.asarray(outT).astype(np.float32)
    return np.ascontiguousarray(outT.T).reshape(1, T, C)


if __name__ == "__main__":
    rng = np.random.default_rng(0)
    x = rng.standard_normal((1, T, C)).astype(np.float32)
    wa = (rng.standard_normal((3 * C, C)) * 0.02).astype(np.float32)
    wp = (rng.standard_normal((C, C)) * 0.02).astype(np.float32)
    y = kernel(x, wa, wp)
    print("kernel out", y.shape, y.dtype, float(np.abs(y).max()))



# revision 40
# speedup vs baseline: 1.1630x; 1.1630x over previous
"""Causal self-attention with RoPE on 8 Trainium2 NeuronCores.

Strategy (tensor-parallel over heads, SPMD-uniform, collective-free):
  - 12 heads -> 8 cores x 2 head slots (4 slots get zero weights).
  - Per core: QKV projection for its 2 heads in [channel, seq] layout;
    RoPE via 3 DVE tensor-tensor ops with sign-folded cos/sin tables;
    causal flash-style attention with scores kept transposed
    (S^T[keys, queries]) so P tiles feed the AV matmul directly; softmax
    denominators via a ones-column in V; per-head normalization; partial
    output projection through the core's slice of w_proj columns.
  - QKV chunk J is interleaved with attention chunk J (causality means
    chunk J only attends keys < 512(J+1)), keeping ScalarE (exp) busy
    from the start.
  - The 8-way sum of partial projections runs on-device: each chunk's
    [C, 512] fp16 partial is ReduceScattered across the cores (overlapped
    with the next chunk's compute), so each core ships back only its
    disjoint [C/8, T] slice.  Host concatenates + transposes.  All
    matmuls run in float32r (TF32-like).
  - Dispatch uses fast_dispatch_compile (no-effect C++ path) on a
    jitted shard_map over the 8 axon-tunneled cores.
"""

import sys

sys.path.insert(0, "/opt/trn_rl_repo")

import numpy as np

import concourse.bass as bass
import concourse.mybir as mybir
import concourse.tile as tile
from concourse import bacc, bass2jax
from concourse.masks import make_identity

FP32 = mybir.dt.float32
FP32R = mybir.dt.float32r
FP16 = mybir.dt.float16
AF = mybir.ActivationFunctionType
ALU = mybir.AluOpType

T = 4096
C = 768
D = 64
N_HEAD = 12
N_CORES = 8
CHUNK = 512          # query chunk (matmul free dim)
NCHUNK = T // CHUNK  # 8
KT = 128             # key tile
ROPE_BASE = 10000.0

# core -> (head_slot_a, head_slot_b); None = zero slot
HEAD_MAP = [(0, 8), (1, 9), (2, 10), (3, 11),
            (4, None), (5, None), (6, None), (7, None)]

_PROG = None  # cached compiled program


def build_program():
    """Build + compile the per-core Bass program (identical on all cores)."""
    nc = bacc.Bacc("TRN2", target_bir_lowering=False, debug=False,
                   num_devices=N_CORES)

    xT_d = nc.dram_tensor("xT", [C, T], FP32R, kind="ExternalInput").ap()
    wqk_u_d = nc.dram_tensor("wqk_u", [C, 256], FP32R, kind="ExternalInput").ap()
    wqk_w_d = nc.dram_tensor("wqk_w", [C, 256], FP32R, kind="ExternalInput").ap()
    w_v_d = nc.dram_tensor("w_v", [C, 128], FP32R, kind="ExternalInput").ap()
    w_pT_d = nc.dram_tensor("w_projT", [128, C], FP32R, kind="ExternalInput").ap()
    cos_d = nc.dram_tensor("rope_cos", [128, T], FP32, kind="ExternalInput").ap()
    sin_d = nc.dram_tensor("rope_sin", [128, T], FP32, kind="ExternalInput").ap()
    mask_d = nc.dram_tensor("masks", [128, 4 * CHUNK], FP32R, kind="ExternalInput").ap()
    CS = C // N_CORES  # 96 rows of the reduced output owned by this core
    out_d = nc.dram_tensor("outT", [CS, T], FP16, kind="ExternalOutput").ap()

    with tile.TileContext(nc) as tc:
        with (
            tc.tile_pool(name="persist", bufs=1) as pers,
            tc.tile_pool(name="xin", bufs=2) as xin,
            tc.tile_pool(name="tmp", bufs=3) as tmps,
            tc.tile_pool(name="ptile", bufs=6) as ptile,
            tc.tile_pool(name="ostage", bufs=3) as ostage,
            tc.tile_pool(name="small", bufs=4) as small,
            tc.tile_pool(name="dram", bufs=1, space="DRAM") as dram,
            tc.tile_pool(name="psUW", bufs=2, space="PSUM") as psUW,  # qkv accumulators
            tc.tile_pool(name="psS", bufs=3, space="PSUM") as psS,    # score tiles
            tc.tile_pool(name="psY", bufs=1, space="PSUM") as psY,    # y accum
            tc.tile_pool(name="psX", bufs=1, space="PSUM") as psX,    # aux (pb/tr) + o
        ):
            # per-chunk partial projections + reduce-scatter bounce buffers
            part = dram.tile([NCHUNK, C, CHUNK], FP16)
            rs_out = dram.tile([NCHUNK, CS, CHUNK], FP16)
            # ---- persistent SBUF ----
            wqk_u = pers.tile([128, 6, 256], FP32R)
            wqk_w = pers.tile([128, 6, 256], FP32R)
            w_v = pers.tile([128, 6, 128], FP32R)
            w_pT = pers.tile([128, C], FP32R)

            mask_sb = pers.tile([128, 4 * CHUNK], FP32R)
            QT = pers.tile([128, T], FP32R)   # rows 0-63 head A, 64-127 head B
            KTt = pers.tile([128, T], FP32R)
            V = pers.tile([128, 32, 130], FP32R)  # [key%128, keytile, vA|1|vB|1]
            Y = pers.tile([128, T], FP32R)    # normalized attention out [ych, q]
            ones_sb = pers.tile([128, D], FP32R)
            ident = pers.tile([128, 128], FP32)
            make_identity(nc, ident[:])

            nc.sync.dma_start(wqk_u[:], wqk_u_d.rearrange("(o p) m -> p o m", p=128))
            nc.gpsimd.dma_start(w_v[:], w_v_d.rearrange("(o p) m -> p o m", p=128))
            nc.gpsimd.dma_start(w_pT[:], w_pT_d[:])
            nc.gpsimd.dma_start(mask_sb[:], mask_d[:])
            ones_f32 = pers.tile([128, D], FP32)
            nc.any.memset(ones_f32[:], 1.0)
            # HAM warm-up: keep PE busy during the initial input DMAs so the
            # clock gate reaches 8/8 before the first real matmuls (results
            # discarded; the tiny copy keeps DCE from dropping the chain)
            warm_ps = psS.tile([128, 128], FP32, tag="s")
            for i in range(16):
                nc.tensor.matmul(warm_ps[:], ident[:, 0:128], ident[:, 0:128],
                                 start=True, stop=True)
            warm_sb = small.tile([1, 8], FP32, tag="warm")
            nc.vector.tensor_copy(warm_sb[:], warm_ps[0:1, 0:8])
            nc.vector.tensor_copy(ones_sb[:], ones_f32[:])
            nc.vector.tensor_copy(V[:, :, 64], ones_sb[:, 0:32])
            nc.vector.tensor_copy(V[:, :, 129], ones_sb[:, 0:32])

            def emit_qkv(J):
                cols = slice(J * CHUNK, (J + 1) * CHUNK)
                xt = xin.tile([128, 6, CHUNK], FP32R)
                xT_r = xT_d.rearrange("(o p) n -> p o n", p=128)
                nc.sync.dma_start(xt[:, 0:3, :], xT_r[:, 0:3, cols])
                nc.sync.dma_start(xt[:, 3:6, :], xT_r[:, 3:6, cols])
                if J == 0:
                    nc.sync.dma_start(
                        wqk_w[:], wqk_w_d.rearrange("(o p) m -> p o m", p=128))
                cs_sb = tmps.tile([128, CHUNK], FP32, tag="cs")
                sn_sb = tmps.tile([128, CHUNK], FP32, tag="sn")
                nc.sync.dma_start(cs_sb[:], cos_d[:, cols])
                nc.sync.dma_start(sn_sb[:], sin_d[:, cols])
                for qk, tgt in ((0, QT), (1, KTt)):
                    u_ps = psUW.tile([128, CHUNK], FP32, tag="uw")
                    w_ps = psUW.tile([128, CHUNK], FP32, tag="uw")
                    wcol = slice(qk * 128, qk * 128 + 128)
                    for k in range(6):
                        nc.tensor.matmul(u_ps[:], wqk_u[:, k, wcol], xt[:, k, :],
                                         start=(k == 0), stop=(k == 5))
                    for k in range(6):
                        nc.tensor.matmul(w_ps[:], wqk_w[:, k, wcol], xt[:, k, :],
                                         start=(k == 0), stop=(k == 5))
                    tm = tmps.tile([128, CHUNK], FP32R, tag="ropetmp")
                    nc.vector.tensor_tensor(tgt[:, cols], u_ps[:], cs_sb[:], ALU.mult)
                    nc.vector.tensor_tensor(tm[:], w_ps[:], sn_sb[:], ALU.mult)
                    nc.vector.tensor_tensor(tgt[:, cols], tgt[:, cols], tm[:], ALU.add)
                # v in [ch, seq] (N=512 full-rate), then PE-transpose per key tile
                v_ps = psUW.tile([128, CHUNK], FP32, tag="uw")
                for k in range(6):
                    nc.tensor.matmul(v_ps[:], w_v[:, k, :], xt[:, k, :],
                                     start=(k == 0), stop=(k == 5))
                vT_sb = tmps.tile([128, CHUNK], FP32, tag="vt")
                nc.vector.tensor_copy(vT_sb[:], v_ps[:])
                for s in range(4):
                    kt_idx = 4 * J + s
                    tr_ps = psX.tile([128, 128], FP32, tag="aux")
                    nc.tensor.transpose(tr_ps[:], vT_sb[:, s * 128:(s + 1) * 128], ident)
                    nc.vector.tensor_copy(V[:, kt_idx, 0:64], tr_ps[:, 0:64])
                    nc.vector.tensor_copy(V[:, kt_idx, 65:129], tr_ps[:, 64:128])

            def emit_att(J):
                cols = slice(J * CHUNK, (J + 1) * CHUNK)
                nkt = 4 * J + 4
                for h in range(2):
                    hsl = slice(64 * h, 64 * h + 64)
                    vsl = slice(65 * h, 65 * h + 65)
                    y_ps = psY.tile([65, CHUNK], FP32, tag="y")
                    for t in range(nkt):
                        d = t - 4 * J
                        qlo = max(0, 128 * d)   # cols < qlo have no valid keys in tile t
                        sub = slice(qlo, CHUNK)
                        qsub = slice(J * CHUNK + qlo, (J + 1) * CHUNK)
                        s_ps = psS.tile([128, CHUNK], FP32, tag="s")
                        nc.tensor.matmul(
                            s_ps[:, sub], KTt[hsl, t * KT:(t + 1) * KT], QT[hsl, qsub],
                            start=True, stop=True)
                        p_sb = ptile.tile([128, CHUNK], FP32R, tag="p")
                        nc.scalar.activation(p_sb[:, sub], s_ps[:, sub], AF.Exp, scale=0.125)
                        if d >= 0:
                            # only qq in [qlo, qlo+128) straddles the causal
                            # boundary; columns beyond are fully valid
                            msub = slice(qlo, qlo + KT)
                            nc.vector.tensor_tensor(
                                p_sb[:, msub], p_sb[:, msub],
                                mask_sb[:, d * CHUNK + qlo:d * CHUNK + qlo + KT],
                                ALU.mult)
                        nc.tensor.matmul(
                            y_ps[:, sub], V[:, t, vsl], p_sb[:, sub],
                            start=(t == 0), stop=(t == nkt - 1))
                    rc = small.tile([1, CHUNK], FP32R, tag="rc")
                    with nc.allow_low_precision(reason="f32r recip for softmax denom"):
                        nc.vector.reciprocal(rc[0:1, :], y_ps[64:65, :])
                    rb = small.tile([64, CHUNK], FP32R, tag="rb")
                    nc.gpsimd.partition_broadcast(rb[:], rc[0:1, :])
                    nc.vector.tensor_tensor(Y[hsl, cols], y_ps[0:64, :], rb[:], ALU.mult)
                for m in range(6):
                    # last chunk: transposes are done, so borrow the aux slot
                    # to double-buffer the projection psum
                    otag = "aux" if (J == NCHUNK - 1 and m % 2) else "o"
                    o_ps = psX.tile([128, CHUNK], FP32, tag=otag)
                    nc.tensor.matmul(o_ps[:], w_pT[:, m * 128:(m + 1) * 128],
                                     Y[:, cols], start=True, stop=True)
                    o_sb = ostage.tile([128, CHUNK], FP16, tag="osb")
                    nc.vector.tensor_copy(o_sb[:], o_ps[:])
                    nc.sync.dma_start(part[J, m * 128:(m + 1) * 128, :], o_sb[:])
                # 8-way reduce-scatter of this chunk's [C, CHUNK] partial:
                # rank r keeps rows [96r, 96r+96).  Runs on the collective
                # engine, overlapped with the next chunk's compute.
                nc.gpsimd.collective_compute(
                    "ReduceScatter",
                    mybir.AluOpType.add,
                    replica_groups=[list(range(N_CORES))],
                    ins=[part[J].opt()],
                    outs=[rs_out[J].opt()],
                )
                nc.sync.dma_start(out_d[:, cols], rs_out[J])

            # attention J emitted right after its QKV; later QKV fills PE idle
            for J in range(NCHUNK):
                emit_qkv(J)
                emit_att(J)

    nc.compile()
    return nc


def _rope_tables():
    theta = 1.0 / (ROPE_BASE ** (np.arange(0, D, 2, dtype=np.float32) / D))  # [32]
    freqs = np.arange(T, dtype=np.float32)[None, :] * theta[:, None]  # [32, T]
    cos32 = np.cos(freqs).astype(np.float32)
    sin32 = np.sin(freqs).astype(np.float32)
    cos128 = np.tile(cos32, (4, 1))
    sin128 = np.concatenate([-sin32, sin32, -sin32, sin32], axis=0)
    return cos128, sin128


def _masks():
    m = np.zeros((128, 4 * CHUNK), dtype=np.float32)
    kk = np.arange(128)[:, None]
    qq = np.arange(CHUNK)[None, :]
    for d in range(4):
        m[:, d * CHUNK:(d + 1) * CHUNK] = (128 * d + kk <= qq).astype(np.float32)
    return m


def _swap_halves(w):
    # w: [rows multiple of 64, C]; swap 32-row halves within each 64 block
    r = w.reshape(-1, 2, 32, w.shape[-1])
    return r[:, ::-1].reshape(w.shape)


def make_in_maps(x, w_attn, w_proj):
    xT = np.ascontiguousarray(x.reshape(T, C).T)  # [C, T]
    cos128, sin128 = _rope_tables()
    masks = _masks()
    in_maps = []
    for c in range(N_CORES):
        qk_rows = []   # rows of w_attn for [qA, qB, kA, kB]
        v_rows = []    # [vA, vB]
        p_cols = []    # w_proj columns for [A(64), B(64)]
        sel = HEAD_MAP[c]
        for part_base in (0, C):  # q rows then k rows
            for h in sel:
                if h is None:
                    qk_rows.append(np.zeros((64, C), np.float32))
                else:
                    qk_rows.append(w_attn[part_base + 64 * h: part_base + 64 * h + 64])
        for h in sel:
            if h is None:
                v_rows.append(np.zeros((64, C), np.float32))
                p_cols.append(np.zeros((C, 64), np.float32))
            else:
                v_rows.append(w_attn[2 * C + 64 * h: 2 * C + 64 * h + 64])
                p_cols.append(w_proj[:, 64 * h: 64 * h + 64])
        qk = np.concatenate(qk_rows, axis=0)          # [256, C]
        wqk_u = np.ascontiguousarray(qk.T)            # [C, 256]
        wqk_w = np.ascontiguousarray(_swap_halves(qk).T)
        w_v = np.ascontiguousarray(np.concatenate(v_rows, axis=0).T)  # [C, 128]
        w_pT = np.ascontiguousarray(np.concatenate(p_cols, axis=1).T)  # [128, C]
        in_maps.append({
            "xT": xT, "wqk_u": wqk_u, "wqk_w": wqk_w, "w_v": w_v,
            "w_projT": w_pT, "rope_cos": cos128, "rope_sin": sin128,
            "masks": masks,
        })
    return in_maps


_RUNNER = None  # cached (jitted fn, metadata)


def build_runner(nc):
    """Jitted SPMD dispatch of the bass NEFF on 8 cores.

    The NEFF itself ends with an 8-way ReduceScatter, so each core returns
    only its disjoint [C/8, T] slice of the summed projection (12.6 MB total
    shipped back instead of 100 MB of per-core partials).
    """
    import jax
    from jax.sharding import Mesh, PartitionSpec
    from jax.experimental.shard_map import shard_map
    from concourse.bass2jax import _bass_exec_p, install_neuronx_cc_hook

    install_neuronx_cc_hook()
    partition_name = nc.partition_id_tensor.name if nc.partition_id_tensor else None
    in_names, in_shapes, out_names, out_avals, zero_outs = [], [], [], [], []
    for alloc in nc.m.functions[0].allocations:
        if not isinstance(alloc, mybir.MemoryLocationSet):
            continue
        name = alloc.memorylocations[0].name
        if alloc.kind == "ExternalInput":
            if name != partition_name:
                in_names.append(name)
                in_shapes.append((tuple(alloc.tensor_shape),
                                  mybir.dt.np(alloc.dtype)))
        elif alloc.kind == "ExternalOutput":
            np_dtype = mybir.dt.np(alloc.dtype)
            out_names.append(name)
            out_avals.append(jax.core.ShapedArray(tuple(alloc.tensor_shape),
                                                  np_dtype))
            zero_outs.append(np.zeros(tuple(alloc.tensor_shape), np_dtype))
    n_params = len(in_names)
    all_in_names = list(in_names) + list(out_names)
    if partition_name is not None:
        all_in_names.append(partition_name)

    def _body(*args):
        operands = list(args)
        if partition_name is not None:
            operands.append(bass2jax.partition_id_tensor())
        outs = _bass_exec_p.bind(
            *operands,
            out_avals=tuple(out_avals),
            in_names=tuple(all_in_names),
            out_names=tuple(out_names),
            lowering_input_output_aliases=(),
            sim_require_finite=True,
            sim_require_nnan=True,
            nc=nc,
        )
        return tuple(outs)

    devices = jax.devices()[:N_CORES]
    mesh = Mesh(np.asarray(devices), ("core",))
    in_specs = (PartitionSpec("core"),) * (n_params + len(out_names))
    sharding = jax.sharding.NamedSharding(mesh, PartitionSpec("core"))

    # Compile with the bass effect suppressed so dispatch takes JAX's C++
    # fast path (fast_dispatch_compile re-registers the error safety net).
    # Argument order matches put_args: inputs (in_names order), then the
    # zero-initialized output buffers.
    arg_structs = [
        jax.ShapeDtypeStruct((N_CORES * shape[0], *shape[1:]), dt,
                             sharding=sharding)
        for shape, dt in in_shapes
    ] + [
        jax.ShapeDtypeStruct((N_CORES * z.shape[0], *z.shape[1:]), z.dtype,
                             sharding=sharding)
        for z in zero_outs
    ]

    def compile_fn():
        jitted = jax.jit(
            shard_map(_body, mesh=mesh, in_specs=in_specs,
                      out_specs=PartitionSpec("core"), check_rep=False),
            keep_unused=True,
        )
        return jitted.lower(*arg_structs).compile()

    sharded = bass2jax.fast_dispatch_compile(compile_fn)

    def put_args(in_maps):
        import jax as _jax
        concat_in = [
            np.concatenate([np.asarray(in_maps[c][nm]) for c in range(N_CORES)],
                           axis=0)
            for nm in in_names
        ]
        concat_zero = [np.zeros((N_CORES * z.shape[0], *z.shape[1:]), z.dtype)
                       for z in zero_outs]
        return [_jax.device_put(a, sharding) for a in concat_in + concat_zero]

    return sharded, put_args


def get_runner():
    global _PROG, _RUNNER
    if _PROG is None:
        _PROG = build_program()
    if _RUNNER is None:
        _RUNNER = build_runner(_PROG)
    return _PROG, _RUNNER


def kernel(x, w_attn, w_proj):
    x = np.asarray(x, dtype=np.float32)
    w_attn = np.asarray(w_attn, dtype=np.float32)
    w_proj = np.asarray(w_proj, dtype=np.float32)
    _, (sharded, put_args) = get_runner()
    in_maps = make_in_maps(x, w_attn, w_proj)
    dev_args = put_args(in_maps)
    (outT,) = sharded(*dev_args)  # [C, T] fp16, already summed across cores
    outT = np.asarray(outT).astype(np.float32)
    return np.ascontiguousarray(outT.T).reshape(1, T, C)


if __name__ == "__main__":
    rng = np.random.default_rng(0)
    x = rng.standard_normal((1, T, C)).astype(np.float32)
    wa = (rng.standard_normal((3 * C, C)) * 0.02).astype(np.float32)
    wp = (rng.standard_normal((C, C)) * 0.02).astype(np.float32)
    y = kernel(x, wa, wp)
    print("kernel out", y.shape, y.dtype, float(np.abs(y).max()))

